# revision 31
# baseline (speedup 1.0000x reference)
"""Trainium2 Bass kernel: sparse (masked) attention with L2 row-normalization.

Per batch b (reference semantics, fp32):
    q = x @ Wq.T ; k = x @ Wk.T ; v = x @ Wv.T          # x: [N, D]
    rel[n, m] = (q[n] . k[m]) * adjacency[m, n]          # multiplicative mask
    out[n]    = sum_m rel[n, m] / ||rel[n, :]||_2 * v[m]

Sharding: data-parallel over batch B=8 -> one batch per NeuronCore, no
collectives. adjacency/weights replicated.

Per-core strategy:
  - q k^T == x (Wq^T Wk) x^T, so the host precomputes G = Wq^T @ Wk and the
    kernel runs ONE projection instead of separate q/k projections; scores are
    computed transposed (S^T[m, n]) so the mask is adjacency in NATIVE layout
    and the AV matmul needs no transposes.
  - scores + projections run in bf16 (fp32 PSUM accumulate).
  - THE AV CHAIN RUNS IN FP8 (e4m3) WITH MatmulPerfMode.DoubleRow: each AV
    matmul contracts K=256 (two key-tiles per instruction), halving the AV
    instruction count vs bf16. The masked scores are written to fp8 by the
    very same DVE mask-multiply that existed anyway (output dtype fp8), and
    v by the same PSUM->SBUF cast, so operand quantization is free.
  - the mask is host-prescaled to adjacency/16 so |st8| stays in fp8 range
    comfortably; the 16s cancel exactly through the L2 normalization
    (rcp = 1/sqrt(sum st8^2) applied to pav = st8^T v8 reproduces
    masked^T v / ||masked||), so no extra scaling instructions exist.
  - row sum-of-squares (a partition-dim reduction) via bf16 ones-vector
    matmuls; the 4 chunk accumulators share ONE PSUM bank at 32-aligned
    partition offsets (tile_position col-groups; dual-fp8 matmuls must write
    partition 0 so they cannot use this trick) and batches are emitted 4
    m-tiles late so the DVE-mask -> ACT-square chain never stalls the PE.
  - inputs load as one tile per 128KB stripe (each matmul gates only on its
    own stripe) issued need-order round-robin over the sync/scalar/gpsimd DMA
    queues; dummy warm-up matmuls keep the PE's HAM clock at full rate
    through the initial DMA wait.
  - 1/||row|| applied as a per-partition scale on the AV output tiles,
    alternating ACT/DVE (stores alternating gpsimd/sync rings) so PSUM banks
    drain two tiles at a time; the final tile quarters across both engines
    and three rings to minimize the tail drain.
"""

from contextlib import ExitStack

import numpy as np
import ml_dtypes

B, N, D = 8, 2048, 512
P = 128  # SBUF partitions
CHUNK = 512  # fp32 free-dim elems per PSUM bank

_cached = {}


def _build(n=N, d=D):
    import concourse.bacc as bacc
    import concourse.mybir as mybir
    import concourse.tile as tile

    f32 = mybir.dt.float32
    bf16 = mybir.dt.bfloat16
    f8 = mybir.dt.float8e4
    DR = mybir.MatmulPerfMode.DoubleRow

    nt = n // P  # key/query 128-tiles
    npair = nt // 2  # key-tile pairs (fp8 DoubleRow granularity)
    dt = d // P  # feature 128-tiles
    ch = min(CHUNK, n)  # free-dim chunk size
    nch = n // ch  # chunks over n
    tpc = ch // P  # 128-tiles per chunk

    nc = bacc.Bacc("TRN2", target_bir_lowering=False, debug=False, num_devices=B)

    # host-prepacked: xTp[p, c, t, j] = x.T[t*P+p, c*ch+j]
    xT_h = nc.dram_tensor("xTp", [P, nch, dt, ch], bf16, kind="ExternalInput")
    # host-prepacked: w2[p, 0, t, e] = G[t*P+p, e] (G = Wq.T @ Wk),
    #                 w2[p, 1, t, e] = Wv.T[t*P+p, e]
    w2_h = nc.dram_tensor("w2", [P, 2, dt, d], bf16, kind="ExternalInput")
    # host-prescaled: adjacency / 16 in fp8 (values 0 or 1/16, both exact)
    adj_h = nc.dram_tensor("adj", [n, n], f8, kind="ExternalInput")
    # bf16 stores halve the output drain; host upcasts to fp32 (adds <=2^-9
    # relative rounding, negligible vs the fp8-AV error budget)
    out_h = nc.dram_tensor("out", [n, d], bf16, kind="ExternalOutput")

    with tile.TileContext(nc) as tc, ExitStack() as ctx:
        sb = ctx.enter_context(tc.tile_pool(name="sb", bufs=1))
        adj_pool = ctx.enter_context(tc.tile_pool(name="adjp", bufs=4))
        outp = ctx.enter_context(tc.tile_pool(name="outp", bufs=8))
        psum = ctx.enter_context(tc.tile_pool(name="psum", bufs=7, space="PSUM"))
        pnrm_pool = ctx.enter_context(tc.tile_pool(name="pnrm", bufs=1, space="PSUM"))

        # ---- input loads ------------------------------------------------
        # one tile per 128KB stripe so each matmul gates only on ITS stripe;
        # stripes issue in need-order, round-robin over the 3 DMA queues
        g_dd = [sb.tile([P, d], bf16, name=f"g{dd}", tag=f"g{dd}") for dd in range(dt)]
        xT_cd = [
            [
                sb.tile([P, ch], bf16, name=f"xT{c}_{dd}", tag=f"xT{c}_{dd}")
                for dd in range(dt)
            ]
            for c in range(nch)
        ]
        wv_e = [
            sb.tile([P, d], bf16, name=f"wv{e}", tag=f"wv{e}") for e in range(dt)
        ]
        loads = []
        for dd in range(dt):  # head-critical: G + x^T chunk 0, paired
            loads.append((g_dd[dd], w2_h[:, 0, dd]))
            loads.append((xT_cd[0][dd], xT_h[:, 0, dd]))
        for e in range(dt):
            loads.append((wv_e[e], w2_h[:, 1, e]))
        for c in range(1, nch):
            for dd in range(dt):
                loads.append((xT_cd[c][dd], xT_h[:, c, dd]))
        rings = [nc.sync, nc.scalar, nc.gpsimd]
        # keep head-critical pairs on one ring each; round-robin the rest
        ring_order = [0, 0, 1, 1, 2, 2, 0, 1] + [
            (2 + i) % 3 for i in range(len(loads) - 8)
        ]
        for (t, src), r in zip(loads, ring_order):
            rings[r].dma_start(t[:], src)

        ones = sb.tile([P, 1], bf16, name="ones", tag="ones")
        nc.vector.memset(ones[:], 1.0)

        # PE warm-up during the initial DMA wait
        warm_rhs = sb.tile([P, ch], bf16, name="warm_rhs", tag="warm_rhs")
        nc.vector.memset(warm_rhs[:], 0.0)
        warm_ps = psum.tile([P, ch], f32, name="mm", tag="mm")
        for _ in range(10):
            nc.tensor.matmul(warm_ps[0:1, :], ones[:], warm_rhs[:])

        def xT_slice(e, m):
            # [128, 128] x^T block: feature-stripe e, key-tile m columns
            return xT_cd[m // tpc][e][:, (m % tpc) * P : (m % tpc + 1) * P]

        # ---- projections, chunk-outer so each xT chunk DMA unlocks work ---
        # xgT[e, n] = sum_d G[d, e] xT[d, n]; v[m, d] = sum_e x[m, e] Wv.T[e, d]
        xgT_sb = [
            sb.tile([P, n], bf16, name=f"xgT{e}", tag=f"xgT{e}") for e in range(dt)
        ]
        # v in fp8, pair-tiles so DoubleRow AV can address two key-tiles at once
        v_pair = [
            sb.tile([P, 2, d], f8, name=f"v{k}", tag=f"v{k}") for k in range(npair)
        ]
        for c in range(nch):
            for e in range(dt):
                pt = psum.tile([P, ch], f32, name="mm", tag="mm")
                for dd in range(dt):
                    nc.tensor.matmul(
                        pt[:],
                        g_dd[dd][:, e * P : (e + 1) * P],
                        xT_cd[c][dd][:],
                        start=(dd == 0),
                        stop=(dd == dt - 1),
                    )
                nc.vector.tensor_copy(xgT_sb[e][:, c * ch : (c + 1) * ch], pt[:])
            for m in range(c * tpc, (c + 1) * tpc):
                pt = psum.tile([P, d], f32, name="mm", tag="mm")
                for e in range(dt):
                    nc.tensor.matmul(
                        pt[:],
                        xT_slice(e, m),
                        wv_e[e][:],
                        start=(e == 0),
                        stop=(e == dt - 1),
                    )
                nc.vector.tensor_copy(v_pair[m // 2][:, m % 2, :], pt[:])

        # ---- scores + mask(fp8) + sum-of-squares -------------------------
        st_pair = [
            sb.tile([P, 2, n], f8, name=f"st{k}", tag=f"st{k}") for k in range(npair)
        ]
        sq_pool = ctx.enter_context(tc.tile_pool(name="sqp", bufs=24))
        # all nch norm accumulators share ONE PSUM bank at partition 32*c
        pnrm = pnrm_pool.tile([P, ch], f32, name="pnrm", tag="pnrm")

        # norm matmuls are emitted in delayed batches so the PE pipeline
        # never waits on the mask/square chain
        sq_tiles = {}

        def emit_norm_batch(mm_idx):
            for c in range(nch):
                nc.tensor.matmul(
                    pnrm[32 * c : 32 * c + 1, :],
                    ones[:],
                    sq_tiles.pop((mm_idx, c))[:],
                    start=(mm_idx == 0),
                    stop=(mm_idx == nt - 1),
                    tile_position=(0, 32 * c),
                )

        for m in range(nt):
            adj_t = adj_pool.tile([P, n], f8, name="adj_t", tag="adj_t")
            nc.sync.dma_start(adj_t[:], adj_h[m * P : (m + 1) * P, :])
            # e-outer: one LDWEIGHTS per stationary, 4 chunk matmuls each
            pss = [psum.tile([P, ch], f32, name="mm", tag="mm") for _ in range(nch)]
            for e in range(dt):
                for c in range(nch):
                    nc.tensor.matmul(
                        pss[c][:],
                        xT_slice(e, m),
                        xgT_sb[e][:, c * ch : (c + 1) * ch],
                        start=(e == 0),
                        stop=(e == dt - 1),
                    )
            k, slot = m // 2, m % 2
            for c in range(nch):
                stm = st_pair[k][:, slot, c * ch : (c + 1) * ch]
                nc.vector.tensor_mul(stm, pss[c][:], adj_t[:, c * ch : (c + 1) * ch])
                sq = sq_pool.tile([P, ch], bf16, name="sq", tag="sq")
                nc.scalar.square(sq[:], stm)
                sq_tiles[m, c] = sq
            if m >= 4 and m % 4 == 0:
                for mm_i in range(m - 4, m):
                    emit_norm_batch(mm_i)
        # first AV groups only need the masked scores, not the squares: emit
        # them ahead of the final norm batches to fill the boundary bubble
        # (their normalization scales are deferred until rcp is ready)
        n_early = min(4, nt)

        def emit_av(t):
            pav = psum.tile([P, d], f32, name="mm", tag="mm")
            for k in range(npair):
                nc.tensor.matmul(
                    pav[:],
                    st_pair[k][:, :, t * P : (t + 1) * P],
                    v_pair[k][:],
                    start=(k == 0),
                    stop=(k == npair - 1),
                    perf_mode=DR,
                )
            return pav

        pav_early = [emit_av(t) for t in range(n_early)]

        for mm_i in range(nt - 4, nt):
            emit_norm_batch(mm_i)

        # ---- 1/||row||: sqrt rows (ACT) -> scatter -> per-chunk recip ----
        # sqrts issue back-to-back on ACT; scatters ride sync/gpsimd only so
        # neither the ACT nor DVE scale stream below queues behind them
        nrm_rows = []
        for c in range(nch):
            nrm_row = sb.tile([1, ch], f32, name=f"nrm_row{c}", tag=f"nrm_row{c}")
            nc.scalar.sqrt(nrm_row[:], pnrm[32 * c : 32 * c + 1, :])
            nrm_rows.append(nrm_row)
        rcp_c = []
        for c in range(nch):
            nrm_tc = sb.tile([P, tpc], f32, name=f"nrm_tc{c}", tag=f"nrm_tc{c}")
            # scatter [1, ch] -> [P, tpc] so scale is a per-partition scalar
            for tt in range(tpc):
                eng = nc.sync if (c * tpc + tt) % 2 == 0 else nc.gpsimd
                eng.dma_start(
                    nrm_tc[:, tt : tt + 1], nrm_rows[c][:, tt * P : (tt + 1) * P]
                )
            rt = sb.tile([P, tpc], f32, name=f"rcp_c{c}", tag=f"rcp_c{c}")
            nc.vector.reciprocal(rt[:], nrm_tc[:])
            rcp_c.append(rt)

        # ---- AV normalization scale + store ------------------------------
        # alternate ACT/DVE for the scale and gpsimd/sync rings for the store
        # so PSUM banks drain two tiles at a time
        def scale_and_store(t, pav):
            ot = outp.tile([P, d], bf16, name="ot", tag="ot")
            rc = rcp_c[t // tpc][:, t % tpc : t % tpc + 1]
            if t % 2 == 0:
                nc.scalar.activation(
                    ot[:], pav[:], mybir.ActivationFunctionType.Copy, scale=rc
                )
                nc.gpsimd.dma_start(out_h[t * P : (t + 1) * P, :], ot[:])
            else:
                nc.vector.tensor_scalar_mul(ot[:], pav[:], rc)
                nc.sync.dma_start(out_h[t * P : (t + 1) * P, :], ot[:])

        for t in range(n_early):
            scale_and_store(t, pav_early[t])
        for t in range(n_early, nt):
            pav = emit_av(t)
            if t == nt - 1:
                # quarter the final scale+store across ACT/DVE and 3 DMA rings
                # so the tail drain runs fully parallel
                rc = rcp_c[t // tpc][:, t % tpc : t % tpc + 1]
                store_eng = [nc.gpsimd, nc.sync, nc.scalar, nc.sync]
                for i in range(4):
                    sl = slice(i * d // 4, (i + 1) * d // 4)
                    oh = outp.tile([P, d // 4], bf16, name=f"oth{i}", tag=f"oth{i}")
                    if i % 2 == 0:
                        nc.scalar.activation(
                            oh[:],
                            pav[:, sl],
                            mybir.ActivationFunctionType.Copy,
                            scale=rc,
                        )
                    else:
                        nc.vector.tensor_scalar_mul(oh[:], pav[:, sl], rc)
                    store_eng[i].dma_start(out_h[t * P : (t + 1) * P, sl], oh[:])
            else:
                scale_and_store(t, pav)

    nc.compile()
    return nc


def _prep_in_maps(inputs, n=N, d=D):
    bf = ml_dtypes.bfloat16
    f8 = ml_dtypes.float8_e4m3
    dt = d // P
    ch = min(CHUNK, n)
    nch = n // ch

    x = np.asarray(inputs["neuron_states"])
    adj = np.ascontiguousarray(
        (np.asarray(inputs["adjacency"]).astype(np.float32) / 16.0).astype(f8)
    )
    G = (
        np.asarray(inputs["Wq"]).T.astype(np.float64)
        @ np.asarray(inputs["Wk"]).astype(np.float64)
    ).astype(np.float32)
    w_all = np.stack([G, np.asarray(inputs["Wv"]).T]).astype(bf)
    # w2[p, i, t, e] = {G, Wv.T}[t*P+p, e]
    w2 = np.ascontiguousarray(w_all.reshape(2, dt, P, d).transpose(2, 0, 1, 3))
    in_maps = []
    for b in range(x.shape[0]):
        xT = x[b].T.astype(bf)  # [d, n]
        # xTp[p, c, t, j] = xT[t*P+p, c*ch+j]
        xTp = np.ascontiguousarray(xT.reshape(dt, P, nch, ch).transpose(1, 2, 0, 3))
        in_maps.append({"xTp": xTp, "adj": adj, "w2": w2})
    return in_maps


def _run(inputs, trace=False, **kw):
    from concourse.bass_utils import run_bass_kernel_spmd

    if "nc" not in _cached:
        _cached["nc"] = _build()
    in_maps = _prep_in_maps(inputs)
    res = run_bass_kernel_spmd(
        _cached["nc"], in_maps, core_ids=list(range(len(in_maps))), trace=trace, **kw
    )
    out = np.stack(
        [np.asarray(r["out"]).astype(np.float32) for r in res.results], axis=0
    )
    return out, res


def kernel(**inputs):
    return _run(inputs)[0]


# revision 34
# speedup vs baseline: 1.0247x; 1.0247x over previous
"""Trainium2 Bass kernel: sparse (masked) attention with L2 row-normalization.

Per batch b (reference semantics, fp32):
    q = x @ Wq.T ; k = x @ Wk.T ; v = x @ Wv.T          # x: [N, D]
    rel[n, m] = (q[n] . k[m]) * adjacency[m, n]          # multiplicative mask
    out[n]    = sum_m rel[n, m] / ||rel[n, :]||_2 * v[m]

Sharding: data-parallel over batch B=8 -> one batch per NeuronCore, no
collectives. adjacency/weights replicated.

Per-core strategy:
  - q k^T == x (Wq^T Wk) x^T, so the host precomputes G = Wq^T @ Wk and the
    kernel runs ONE projection instead of separate q/k projections; scores are
    computed transposed (S^T[m, n]) so the mask is adjacency in NATIVE layout
    and the AV matmul needs no transposes.
  - scores + projections run in bf16 (fp32 PSUM accumulate).
  - THE AV CHAIN RUNS IN FP8 (e4m3) WITH MatmulPerfMode.DoubleRow: each AV
    matmul contracts K=256 (two key-tiles per instruction), halving the AV
    instruction count vs bf16. The masked scores are written to fp8 by the
    very same DVE mask-multiply that existed anyway (output dtype fp8), and
    v by the same PSUM->SBUF cast, so operand quantization is free.
  - the mask is host-prescaled to adjacency/16 so |st8| stays in fp8 range
    comfortably; the 16s cancel exactly through the L2 normalization
    (rcp = 1/sqrt(sum st8^2) applied to pav = st8^T v8 reproduces
    masked^T v / ||masked||), so no extra scaling instructions exist.
  - row sum-of-squares (a partition-dim reduction) via bf16 ones-vector
    matmuls; the 4 chunk accumulators share ONE PSUM bank at 32-aligned
    partition offsets (tile_position col-groups; dual-fp8 matmuls must write
    partition 0 so they cannot use this trick) and batches are emitted 4
    m-tiles late so the DVE-mask -> ACT-square chain never stalls the PE.
  - inputs load as one tile per 128KB stripe (each matmul gates only on its
    own stripe) issued need-order round-robin over the sync/scalar/gpsimd DMA
    queues; dummy warm-up matmuls keep the PE's HAM clock at full rate
    through the initial DMA wait.
  - 1/||row|| applied as a per-partition scale on the AV output tiles,
    alternating ACT/DVE (stores alternating gpsimd/sync rings) so PSUM banks
    drain two tiles at a time; the final tile quarters across both engines
    and three rings to minimize the tail drain.
"""

from contextlib import ExitStack

import numpy as np
import ml_dtypes

B, N, D = 8, 2048, 512
P = 128  # SBUF partitions
CHUNK = 512  # fp32 free-dim elems per PSUM bank

_cached = {}


def _build(n=N, d=D):
    import concourse.bacc as bacc
    import concourse.mybir as mybir
    import concourse.tile as tile

    f32 = mybir.dt.float32
    bf16 = mybir.dt.bfloat16
    f8 = mybir.dt.float8e4
    DR = mybir.MatmulPerfMode.DoubleRow

    nt = n // P  # key/query 128-tiles
    npair = nt // 2  # key-tile pairs (fp8 DoubleRow granularity)
    dt = d // P  # feature 128-tiles
    ch = min(CHUNK, n)  # free-dim chunk size
    nch = n // ch  # chunks over n
    tpc = ch // P  # 128-tiles per chunk

    nc = bacc.Bacc("TRN2", target_bir_lowering=False, debug=False, num_devices=B)

    # host-prepacked: xTp[p, c, t, j] = x.T[t*P+p, c*ch+j]
    xT_h = nc.dram_tensor("xTp", [P, nch, dt, ch], bf16, kind="ExternalInput")
    # host-prepacked: w2[p, 0, t, e] = G[t*P+p, e] (G = Wq.T @ Wk),
    #                 w2[p, 1, t, e] = Wv.T[t*P+p, e]
    w2_h = nc.dram_tensor("w2", [P, 2, dt, d], bf16, kind="ExternalInput")
    # host-prescaled: adjacency / 16 in fp8 (values 0 or 1/16, both exact)
    adj_h = nc.dram_tensor("adj", [n, n], f8, kind="ExternalInput")
    # bf16 stores halve the output drain; host upcasts to fp32 (adds <=2^-9
    # relative rounding, negligible vs the fp8-AV error budget)
    out_h = nc.dram_tensor("out", [n, d], bf16, kind="ExternalOutput")

    with tile.TileContext(nc) as tc, ExitStack() as ctx:
        sb = ctx.enter_context(tc.tile_pool(name="sb", bufs=1))
        adj_pool = ctx.enter_context(tc.tile_pool(name="adjp", bufs=4))
        outp = ctx.enter_context(tc.tile_pool(name="outp", bufs=8))
        psum = ctx.enter_context(tc.tile_pool(name="psum", bufs=7, space="PSUM"))
        pnrm_pool = ctx.enter_context(tc.tile_pool(name="pnrm", bufs=1, space="PSUM"))

        # ---- input loads ------------------------------------------------
        # one tile per 128KB stripe so each matmul gates only on ITS stripe;
        # stripes issue in need-order, round-robin over the 3 DMA queues
        g_dd = [sb.tile([P, d], bf16, name=f"g{dd}", tag=f"g{dd}") for dd in range(dt)]
        xT_cd = [
            [
                sb.tile([P, ch], bf16, name=f"xT{c}_{dd}", tag=f"xT{c}_{dd}")
                for dd in range(dt)
            ]
            for c in range(nch)
        ]
        wv_e = [
            sb.tile([P, d], bf16, name=f"wv{e}", tag=f"wv{e}") for e in range(dt)
        ]
        loads = []
        for dd in range(dt):  # head-critical: G + x^T chunk 0, paired
            loads.append((g_dd[dd], w2_h[:, 0, dd]))
            loads.append((xT_cd[0][dd], xT_h[:, 0, dd]))
        for e in range(dt):
            loads.append((wv_e[e], w2_h[:, 1, e]))
        for c in range(1, nch):
            for dd in range(dt):
                loads.append((xT_cd[c][dd], xT_h[:, c, dd]))
        rings = [nc.sync, nc.scalar, nc.gpsimd]
        # keep head-critical pairs on one ring each; round-robin the rest
        ring_order = [0, 0, 1, 1, 2, 2, 0, 1] + [
            (2 + i) % 3 for i in range(len(loads) - 8)
        ]
        for (t, src), r in zip(loads, ring_order):
            rings[r].dma_start(t[:], src)

        ones = sb.tile([P, 1], bf16, name="ones", tag="ones")
        nc.vector.memset(ones[:], 1.0)

        # PE warm-up during the initial DMA wait
        warm_rhs = sb.tile([P, ch], bf16, name="warm_rhs", tag="warm_rhs")
        nc.vector.memset(warm_rhs[:], 0.0)
        warm_ps = psum.tile([P, ch], f32, name="mm", tag="mm")
        for _ in range(10):
            nc.tensor.matmul(warm_ps[0:1, :], ones[:], warm_rhs[:])

        def xT_slice(e, m):
            # [128, 128] x^T block: feature-stripe e, key-tile m columns
            return xT_cd[m // tpc][e][:, (m % tpc) * P : (m % tpc + 1) * P]

        # ---- projections, chunk-outer so each xT chunk DMA unlocks work ---
        # xgT[e, n] = sum_d G[d, e] xT[d, n]; v[m, d] = sum_e x[m, e] Wv.T[e, d]
        xgT_sb = [
            sb.tile([P, n], bf16, name=f"xgT{e}", tag=f"xgT{e}") for e in range(dt)
        ]
        # v in fp8, pair-tiles so DoubleRow AV can address two key-tiles at once
        v_pair = [
            sb.tile([P, 2, d], f8, name=f"v{k}", tag=f"v{k}") for k in range(npair)
        ]
        for c in range(nch):
            for e in range(dt):
                pt = psum.tile([P, ch], f32, name="mm", tag="mm")
                for dd in range(dt):
                    nc.tensor.matmul(
                        pt[:],
                        g_dd[dd][:, e * P : (e + 1) * P],
                        xT_cd[c][dd][:],
                        start=(dd == 0),
                        stop=(dd == dt - 1),
                    )
                nc.vector.tensor_copy(xgT_sb[e][:, c * ch : (c + 1) * ch], pt[:])
            for m in range(c * tpc, (c + 1) * tpc):
                pt = psum.tile([P, d], f32, name="mm", tag="mm")
                for e in range(dt):
                    nc.tensor.matmul(
                        pt[:],
                        xT_slice(e, m),
                        wv_e[e][:],
                        start=(e == 0),
                        stop=(e == dt - 1),
                    )
                nc.vector.tensor_copy(v_pair[m // 2][:, m % 2, :], pt[:])

        # ---- scores + mask(fp8) + sum-of-squares -------------------------
        st_pair = [
            sb.tile([P, 2, n], f8, name=f"st{k}", tag=f"st{k}") for k in range(npair)
        ]
        sq_pool = ctx.enter_context(tc.tile_pool(name="sqp", bufs=24))
        # all nch norm accumulators share ONE PSUM bank at partition 32*c
        pnrm = pnrm_pool.tile([P, ch], f32, name="pnrm", tag="pnrm")

        # squares of each key-tile PAIR are summed on DVE (emitted one tile
        # late so the in-order DVE queue never waits on ACT), halving the
        # ones-matmul count; norm matmul batches stay delayed so the PE
        # never waits on the mask/square/add chain
        sq_tiles = {}
        sq_sum = {}

        def emit_pair_add(k):
            for c in range(nch):
                ss = sq_pool.tile([P, ch], bf16, name="ss", tag="ss")
                nc.vector.tensor_add(
                    ss[:], sq_tiles.pop((2 * k, c))[:], sq_tiles.pop((2 * k + 1, c))[:]
                )
                sq_sum[k, c] = ss

        def emit_norm_pair(k):
            for c in range(nch):
                nc.tensor.matmul(
                    pnrm[32 * c : 32 * c + 1, :],
                    ones[:],
                    sq_sum.pop((k, c))[:],
                    start=(k == 0),
                    stop=(k == npair - 1),
                    tile_position=(0, 32 * c),
                )

        for m in range(nt):
            adj_t = adj_pool.tile([P, n], f8, name="adj_t", tag="adj_t")
            nc.sync.dma_start(adj_t[:], adj_h[m * P : (m + 1) * P, :])
            # e-outer: one LDWEIGHTS per stationary, 4 chunk matmuls each
            pss = [psum.tile([P, ch], f32, name="mm", tag="mm") for _ in range(nch)]
            for e in range(dt):
                for c in range(nch):
                    nc.tensor.matmul(
                        pss[c][:],
                        xT_slice(e, m),
                        xgT_sb[e][:, c * ch : (c + 1) * ch],
                        start=(e == 0),
                        stop=(e == dt - 1),
                    )
            k, slot = m // 2, m % 2
            for c in range(nch):
                stm = st_pair[k][:, slot, c * ch : (c + 1) * ch]
                nc.vector.tensor_mul(stm, pss[c][:], adj_t[:, c * ch : (c + 1) * ch])
                sq = sq_pool.tile([P, ch], bf16, name="sq", tag="sq")
                nc.scalar.square(sq[:], stm)
                sq_tiles[m, c] = sq
            if m >= 2 and m % 2 == 0:
                emit_pair_add(m // 2 - 1)
            if m >= 8 and m % 4 == 0:
                emit_norm_pair(m // 2 - 4)
                emit_norm_pair(m // 2 - 3)
        # first AV groups only need the masked scores, not the squares: emit
        # them ahead of the final norm batches to fill the boundary bubble
        # (their normalization scales are deferred until rcp is ready)
        n_early = min(4, nt)

        def emit_av(t):
            pav = psum.tile([P, d], f32, name="mm", tag="mm")
            for k in range(npair):
                nc.tensor.matmul(
                    pav[:],
                    st_pair[k][:, :, t * P : (t + 1) * P],
                    v_pair[k][:],
                    start=(k == 0),
                    stop=(k == npair - 1),
                    perf_mode=DR,
                )
            return pav

        emit_pair_add(npair - 1)
        pav_early = [emit_av(t) for t in range(n_early)]

        for k in range(npair - 4, npair):
            emit_norm_pair(k)

        # ---- 1/||row||: sqrt rows (ACT) -> scatter -> per-chunk recip ----
        # sqrts issue back-to-back on ACT; scatters ride sync/gpsimd only so
        # neither the ACT nor DVE scale stream below queues behind them
        nrm_rows = []
        for c in range(nch):
            nrm_row = sb.tile([1, ch], f32, name=f"nrm_row{c}", tag=f"nrm_row{c}")
            nc.scalar.sqrt(nrm_row[:], pnrm[32 * c : 32 * c + 1, :])
            nrm_rows.append(nrm_row)
        rcp_c = []
        for c in range(nch):
            nrm_tc = sb.tile([P, tpc], f32, name=f"nrm_tc{c}", tag=f"nrm_tc{c}")
            # scatter [1, ch] -> [P, tpc] so scale is a per-partition scalar
            for tt in range(tpc):
                eng = nc.sync if (c * tpc + tt) % 2 == 0 else nc.gpsimd
                eng.dma_start(
                    nrm_tc[:, tt : tt + 1], nrm_rows[c][:, tt * P : (tt + 1) * P]
                )
            rt = sb.tile([P, tpc], f32, name=f"rcp_c{c}", tag=f"rcp_c{c}")
            nc.vector.reciprocal(rt[:], nrm_tc[:])
            rcp_c.append(rt)

        # ---- AV normalization scale + store ------------------------------
        # alternate ACT/DVE for the scale and gpsimd/sync rings for the store
        # so PSUM banks drain two tiles at a time
        def scale_and_store(t, pav):
            ot = outp.tile([P, d], bf16, name="ot", tag="ot")
            rc = rcp_c[t // tpc][:, t % tpc : t % tpc + 1]
            if t % 2 == 0:
                nc.scalar.activation(
                    ot[:], pav[:], mybir.ActivationFunctionType.Copy, scale=rc
                )
                nc.gpsimd.dma_start(out_h[t * P : (t + 1) * P, :], ot[:])
            else:
                nc.vector.tensor_scalar_mul(ot[:], pav[:], rc)
                nc.sync.dma_start(out_h[t * P : (t + 1) * P, :], ot[:])

        for t in range(n_early):
            scale_and_store(t, pav_early[t])
        for t in range(n_early, nt):
            pav = emit_av(t)
            if t == nt - 1:
                # quarter the final scale+store across ACT/DVE and 3 DMA rings
                # so the tail drain runs fully parallel
                rc = rcp_c[t // tpc][:, t % tpc : t % tpc + 1]
                store_eng = [nc.gpsimd, nc.sync, nc.scalar, nc.sync]
                for i in range(4):
                    sl = slice(i * d // 4, (i + 1) * d // 4)
                    oh = outp.tile([P, d // 4], bf16, name=f"oth{i}", tag=f"oth{i}")
                    if i % 2 == 0:
                        nc.scalar.activation(
                            oh[:],
                            pav[:, sl],
                            mybir.ActivationFunctionType.Copy,
                            scale=rc,
                        )
                    else:
                        nc.vector.tensor_scalar_mul(oh[:], pav[:, sl], rc)
                    store_eng[i].dma_start(out_h[t * P : (t + 1) * P, sl], oh[:])
            else:
                scale_and_store(t, pav)

    nc.compile()
    return nc


def _prep_in_maps(inputs, n=N, d=D):
    bf = ml_dtypes.bfloat16
    f8 = ml_dtypes.float8_e4m3
    dt = d // P
    ch = min(CHUNK, n)
    nch = n // ch

    x = np.asarray(inputs["neuron_states"])
    adj = np.ascontiguousarray(
        (np.asarray(inputs["adjacency"]).astype(np.float32) / 16.0).astype(f8)
    )
    G = (
        np.asarray(inputs["Wq"]).T.astype(np.float64)
        @ np.asarray(inputs["Wk"]).astype(np.float64)
    ).astype(np.float32)
    w_all = np.stack([G, np.asarray(inputs["Wv"]).T]).astype(bf)
    # w2[p, i, t, e] = {G, Wv.T}[t*P+p, e]
    w2 = np.ascontiguousarray(w_all.reshape(2, dt, P, d).transpose(2, 0, 1, 3))
    in_maps = []
    for b in range(x.shape[0]):
        xT = x[b].T.astype(bf)  # [d, n]
        # xTp[p, c, t, j] = xT[t*P+p, c*ch+j]
        xTp = np.ascontiguousarray(xT.reshape(dt, P, nch, ch).transpose(1, 2, 0, 3))
        in_maps.append({"xTp": xTp, "adj": adj, "w2": w2})
    return in_maps


def _run(inputs, trace=False, **kw):
    from concourse.bass_utils import run_bass_kernel_spmd

    if "nc" not in _cached:
        _cached["nc"] = _build()
    in_maps = _prep_in_maps(inputs)
    res = run_bass_kernel_spmd(
        _cached["nc"], in_maps, core_ids=list(range(len(in_maps))), trace=trace, **kw
    )
    out = np.stack(
        [np.asarray(r["out"]).astype(np.float32) for r in res.results], axis=0
    )
    return out, res


def kernel(**inputs):
    return _run(inputs)[0]
